# revision 4
# baseline (speedup 1.0000x reference)
"""Trainium2 Bass kernel for nn_Attention_59528246723073.

Reference (per batch b, channel c; x[b,c] is [S=256, T=64]):
    fs = tanh(x @ Wspect[c])            # [S]   (contract T)
    ft = tanh(x.T @ Wtemp[c])           # [T]   (contract S)
    a  = softmax_S(fs) * 100
    g  = softmax_T(ft)
    out[b,c,s,t] = x[b,c,s,t] * a[s] * g[t]

Distribution: data-parallel over batch B=32 -> 4 per core on 8 cores.
Per-core layout: for each local batch b, one SBUF tile [128 part = channels,
16384 free = S*T] (x[b] is exactly this, contiguous).  All reductions run
along the free axis on VectorE; softmaxes are free-axis ops; tanh/exp on
ScalarE with fused bias/accum.
"""

import numpy as np

import concourse.bass as bass
import concourse.tile as tile
from concourse import bacc, mybir
from concourse.bass_utils import run_bass_kernel_spmd

B, C, S, T = 32, 128, 256, 64
N_CORES = 8
B_LOC = B // N_CORES
NCHUNK = 4
SC = S // NCHUNK  # s-values per chunk
F32 = mybir.dt.float32

_NC = None


def build_nc():
    nc = bacc.Bacc("TRN2", target_bir_lowering=False, debug=False)
    x = nc.dram_tensor("x", [B_LOC, C, S, T], F32, kind="ExternalInput")
    ws = nc.dram_tensor("wspect", [C, T], F32, kind="ExternalInput")
    wt = nc.dram_tensor("wtemp", [C, S], F32, kind="ExternalInput")
    out = nc.dram_tensor("out", [B_LOC, C, S, T], F32, kind="ExternalOutput")

    AF = mybir.ActivationFunctionType
    OP = mybir.AluOpType

    with tile.TileContext(nc) as tc:
        with (
            tc.tile_pool(name="consts", bufs=1) as cpool,
            tc.tile_pool(name="xp", bufs=2) as xpool,
            tc.tile_pool(name="tmp", bufs=3) as tpool,
            tc.tile_pool(name="small", bufs=2) as spool,
        ):
            ws_sb = cpool.tile([C, T], F32)
            nc.sync.dma_start(ws_sb[:], ws[:])
            wt_sb = cpool.tile([C, S], F32)
            nc.sync.dma_start(wt_sb[:], wt[:])
            # [128, s-chunk(bcast), t]
            ws_bc = ws_sb.unsqueeze(1).to_broadcast((C, SC, T))

            for b in range(B_LOC):
                X = xpool.tile([C, S * T], F32, tag="X")
                X3 = X.rearrange("p (s t) -> p s t", t=T)
                for k in range(NCHUNK):
                    sl = slice(k * SC, (k + 1) * SC)
                    nc.sync.dma_start(X3[:, sl, :], x[b, :, sl, :])

                fs = spool.tile([C, S], F32, tag="fs")
                ftp = spool.tile([C, NCHUNK * T], F32, tag="ftp")
                for k in range(NCHUNK):
                    sl = slice(k * SC, (k + 1) * SC)
                    # fs[:, sl] = sum_t X[:, sl, :] * Wspect[:, None, :]
                    tmp = tpool.tile([C, SC * T], F32, tag="tmp")
                    t3 = tmp.rearrange("p (s t) -> p s t", t=T)
                    nc.vector.tensor_tensor(t3, X3[:, sl, :], ws_bc, op=OP.mult)
                    nc.vector.reduce_sum(fs[:, sl], t3, axis=mybir.AxisListType.X)
                    # ftp[:, k*T:(k+1)*T] = sum_{s in chunk} X[:, sl, :] * Wtemp[:, sl, None]
                    wt_bc = wt_sb[:, sl].unsqueeze(2).to_broadcast((C, SC, T))
                    tmp2 = tpool.tile([C, SC * T], F32, tag="tmp")
                    t3b = tmp2.rearrange("p (s t) -> p s t", t=T)
                    nc.vector.tensor_tensor(t3b, X3[:, sl, :], wt_bc, op=OP.mult)
                    nc.vector.reduce_sum(
                        ftp[:, k * T : (k + 1) * T],
                        t3b.transpose([0, 2, 1]),
                        axis=mybir.AxisListType.X,
                    )
                ft = spool.tile([C, T], F32, tag="ft")
                nc.vector.reduce_sum(
                    ft[:],
                    ftp.rearrange("p (k t) -> p k t", t=T).transpose([0, 2, 1]),
                    axis=mybir.AxisListType.X,
                )

                nc.scalar.activation(fs[:], fs[:], AF.Tanh)
                nc.scalar.activation(ft[:], ft[:], AF.Tanh)

                # softmax over S (-> a, scaled by 100), softmax over T (-> g)
                nmax = spool.tile([C, 1], F32, tag="nmax")
                ssum = spool.tile([C, 1], F32, tag="ssum")
                rec = spool.tile([C, 1], F32, tag="rec")
                nc.vector.reduce_max(
                    nmax[:], fs[:], axis=mybir.AxisListType.X, negate=True
                )
                nc.scalar.activation(
                    fs[:], fs[:], AF.Exp, bias=nmax[:, 0:1], accum_out=ssum[:, 0:1]
                )
                nc.vector.reciprocal(rec[:], ssum[:])
                nc.vector.tensor_scalar(
                    out=fs[:], in0=fs[:], scalar1=rec[:, 0:1], scalar2=100.0,
                    op0=OP.mult, op1=OP.mult,
                )

                nmax2 = spool.tile([C, 1], F32, tag="nmax2")
                ssum2 = spool.tile([C, 1], F32, tag="ssum2")
                rec2 = spool.tile([C, 1], F32, tag="rec2")
                nc.vector.reduce_max(
                    nmax2[:], ft[:], axis=mybir.AxisListType.X, negate=True
                )
                nc.scalar.activation(
                    ft[:], ft[:], AF.Exp, bias=nmax2[:, 0:1], accum_out=ssum2[:, 0:1]
                )
                nc.vector.reciprocal(rec2[:], ssum2[:])
                nc.vector.tensor_scalar(
                    out=ft[:], in0=ft[:], scalar1=rec2[:, 0:1], scalar2=None,
                    op0=OP.mult,
                )

                g_bc = ft.unsqueeze(1).to_broadcast((C, SC, T))
                for k in range(NCHUNK):
                    sl = slice(k * SC, (k + 1) * SC)
                    a_bc = fs[:, sl].unsqueeze(2).to_broadcast((C, SC, T))
                    oc = tpool.tile([C, SC * T], F32, tag="tmp")
                    o3 = oc.rearrange("p (s t) -> p s t", t=T)
                    nc.vector.tensor_tensor(o3, X3[:, sl, :], a_bc, op=OP.mult)
                    nc.vector.tensor_tensor(o3, o3, g_bc, op=OP.mult)
                    nc.sync.dma_start(out[b, :, sl, :], o3)

    nc.compile()
    return nc


def get_nc():
    global _NC
    if _NC is None:
        _NC = build_nc()
    return _NC


def shard_inputs(x, Wspect, Wtemp):
    ws = np.ascontiguousarray(Wspect.reshape(C, T).astype(np.float32))
    wt = np.ascontiguousarray(Wtemp.reshape(C, S).astype(np.float32))
    x = np.ascontiguousarray(x.astype(np.float32))
    return [
        {"x": x[i * B_LOC : (i + 1) * B_LOC], "wspect": ws, "wtemp": wt}
        for i in range(N_CORES)
    ]


def unshard(results):
    return np.concatenate([r["out"] for r in results], axis=0)


def kernel(x, Wspect, Wtemp):
    nc = get_nc()
    in_maps = shard_inputs(x, Wspect, Wtemp)
    res = run_bass_kernel_spmd(nc, in_maps, core_ids=list(range(N_CORES)))
    return unshard(res.results)


# revision 6
# speedup vs baseline: 1.7028x; 1.7028x over previous
"""Trainium2 Bass kernel for nn_Attention_59528246723073.

Reference (per batch b, channel c; x[b,c] is [S=256, T=64]):
    fs = tanh(x @ Wspect[c])            # [S]   (contract T)
    ft = tanh(x.T @ Wtemp[c])           # [T]   (contract S)
    a  = softmax_S(fs) * 100
    g  = softmax_T(ft)
    out[b,c,s,t] = x[b,c,s,t] * a[s] * g[t]

Distribution: data-parallel over batch B=32 -> 4 per core on 8 cores.

Per-core layout: for each local batch b, SBUF tile [128 part = channels,
S*T free] (x[b] is exactly this, contiguous).  f32 chunks are cast to fp16
on ScalarE; all big elementwise ops run on VectorE in fp16 with the 2x_1p
perf mode (innermost step 1 on every operand):
  - fs-mul multiplies by Wspect broadcast over s (inner t contiguous),
  - ft-mul multiplies by a pre-materialized Wtemp replica (contiguous, flat),
  - ft reduction = flat in-place fold chain over s, accumulated across chunks,
  - fs reduction = one in-place fold over t + one f32 tensor_reduce,
  - final: g-mul (inner-contiguous bcast) then a-mul via a paired-duplicate
    a2[p, 2s+j] = a[p,s] so the broadcast keeps innermost step 1.
Output stays fp16 in SBUF and is cast to f32 by the SWDGE output DMA.
"""

import numpy as np

import concourse.bass as bass
import concourse.tile as tile
from concourse import bacc, mybir
from concourse.bass_utils import run_bass_kernel_spmd

B, C, S, T = 32, 128, 256, 64
N_CORES = 8
B_LOC = B // N_CORES
NCHUNK = 4
SC = S // NCHUNK  # s-values per chunk
F32 = mybir.dt.float32
F16 = mybir.dt.float16

_NC = None


def build_nc():
    nc = bacc.Bacc("TRN2", target_bir_lowering=False, debug=False)
    x = nc.dram_tensor("x", [B_LOC, C, S, T], F32, kind="ExternalInput")
    ws = nc.dram_tensor("wspect", [C, T], F32, kind="ExternalInput")
    wt = nc.dram_tensor("wtemp", [C, S], F32, kind="ExternalInput")
    out = nc.dram_tensor("out", [B_LOC, C, S, T], F32, kind="ExternalOutput")

    AF = mybir.ActivationFunctionType
    OP = mybir.AluOpType
    AX = mybir.AxisListType

    with tile.TileContext(nc) as tc:
        with (
            tc.tile_pool(name="consts", bufs=1) as cpool,
            tc.tile_pool(name="xf", bufs=3) as xfpool,
            tc.tile_pool(name="x2", bufs=2) as x2pool,
            tc.tile_pool(name="tmp", bufs=4) as tpool,
            tc.tile_pool(name="small", bufs=2) as spool,
        ):
            # --- constants: weights in fp16; Wtemp replicated along t ---
            ws_f = cpool.tile([C, T], F32)
            nc.sync.dma_start(ws_f[:], ws[:])
            wt_f = cpool.tile([C, S], F32)
            nc.sync.dma_start(wt_f[:], wt[:])
            ws16 = cpool.tile([C, T], F16)
            nc.scalar.activation(ws16[:], ws_f[:], AF.Copy)
            # wt_rep[c, s, t] = Wtemp[c, s]  (fp16, contiguous)
            wt_rep = cpool.tile([C, S * T], F16)
            wt_rep3 = wt_rep.rearrange("p (s t) -> p s t", t=T)
            nc.scalar.activation(
                wt_rep3, wt_f[:].unsqueeze(2).to_broadcast((C, S, T)), AF.Copy
            )
            # [128, s-chunk(bcast), t] — inner step 1 -> 2x
            ws_bc = ws16.unsqueeze(1).to_broadcast((C, SC, T))

            for b in range(B_LOC):
                X2 = x2pool.tile([C, S * T], F16, tag="X2")
                X23 = X2.rearrange("p (s t) -> p s t", t=T)
                fs = spool.tile([C, S], F32, tag="fs")
                ft = spool.tile([C, T], F32, tag="ft")

                for k in range(NCHUNK):
                    sl = slice(k * SC, (k + 1) * SC)
                    fsl = slice(k * SC * T, (k + 1) * SC * T)
                    # load f32 chunk, cast to fp16 on ScalarE
                    with nc.named_scope("load"):
                        xf = xfpool.tile([C, SC * T], F32, tag="xf")
                        nc.sync.dma_start(xf[:], x[b, :, sl, :])
                        nc.scalar.activation(X2[:, fsl], xf[:], AF.Copy)

                    xc = X23[:, sl, :]
                    xcf = X2[:, fsl]
                    # fs[:, sl] = sum_t xc * Wspect[:, None, :]
                    with nc.named_scope("fs"):
                        tmp = tpool.tile([C, SC * T], F16, tag="tmp")
                        t3 = tmp.rearrange("p (s t) -> p s t", t=T)
                        nc.vector.tensor_tensor(t3, xc, ws_bc, op=OP.mult)
                        nc.vector.tensor_tensor(
                            t3[:, :, 0 : T // 2], t3[:, :, 0 : T // 2],
                            t3[:, :, T // 2 : T], op=OP.add,
                        )
                        nc.vector.reduce_sum(
                            fs[:, sl], t3[:, :, 0 : T // 2], axis=AX.X
                        )
                    # ft += sum_{s in chunk} xc * Wtemp[:, sl, None]
                    # (all-flat fp16 fold chain over s, then tiny accumulate)
                    with nc.named_scope("ft"):
                        tmp2 = tpool.tile([C, SC * T], F16, tag="tmp")
                        nc.vector.tensor_tensor(
                            tmp2[:], xcf, wt_rep[:, fsl], op=OP.mult
                        )
                        w = SC * T // 2
                        while w >= T:
                            nc.vector.tensor_tensor(
                                tmp2[:, 0:w], tmp2[:, 0:w], tmp2[:, w : 2 * w],
                                op=OP.add,
                            )
                            w //= 2
                        if k == 0:
                            nc.vector.tensor_copy(ft[:], tmp2[:, 0:T])
                        else:
                            nc.vector.tensor_tensor(
                                ft[:], ft[:], tmp2[:, 0:T], op=OP.add
                            )

                with nc.named_scope("softmax"):
                    nc.scalar.activation(fs[:], fs[:], AF.Tanh)
                    nc.scalar.activation(ft[:], ft[:], AF.Tanh)

                    # softmax over S -> a2 (fp16, paired-duplicate, x100)
                    nmax = spool.tile([C, 1], F32, tag="nmax")
                    ssum = spool.tile([C, 1], F32, tag="ssum")
                    rec = spool.tile([C, 1], F32, tag="rec")
                    nc.vector.reduce_max(nmax[:], fs[:], axis=AX.X, negate=True)
                    nc.scalar.activation(
                        fs[:], fs[:], AF.Exp, bias=nmax[:, 0:1],
                        accum_out=ssum[:, 0:1],
                    )
                    nc.vector.reciprocal(rec[:], ssum[:])
                    a2 = spool.tile([C, 2 * S], F16, tag="a2")
                    nc.vector.tensor_scalar(
                        out=a2.rearrange("p (s j) -> p s j", j=2),
                        in0=fs[:].unsqueeze(2).to_broadcast((C, S, 2)),
                        scalar1=rec[:, 0:1], scalar2=100.0,
                        op0=OP.mult, op1=OP.mult,
                    )

                    # softmax over T -> g16 (fp16)
                    nmax2 = spool.tile([C, 1], F32, tag="nmax2")
                    ssum2 = spool.tile([C, 1], F32, tag="ssum2")
                    rec2 = spool.tile([C, 1], F32, tag="rec2")
                    nc.vector.reduce_max(nmax2[:], ft[:], axis=AX.X, negate=True)
                    nc.scalar.activation(
                        ft[:], ft[:], AF.Exp, bias=nmax2[:, 0:1],
                        accum_out=ssum2[:, 0:1],
                    )
                    nc.vector.reciprocal(rec2[:], ssum2[:])
                    g16 = spool.tile([C, T], F16, tag="g16")
                    nc.vector.tensor_scalar(
                        out=g16[:], in0=ft[:], scalar1=rec2[:, 0:1], scalar2=None,
                        op0=OP.mult,
                    )
                g_bc = g16.unsqueeze(1).to_broadcast((C, SC, T))

                for k in range(NCHUNK):
                    sl = slice(k * SC, (k + 1) * SC)
                    with nc.named_scope("final"):
                        oc = tpool.tile([C, SC * T], F16, tag="tmp")
                        o3 = oc.rearrange("p (s t) -> p s t", t=T)
                        nc.vector.tensor_tensor(o3, X23[:, sl, :], g_bc, op=OP.mult)
                        # a-mul on fp16 pairs: innermost step-1 j dim keeps 2x
                        oP = oc.rearrange(
                            "p (s pr j) -> p s pr j", pr=T // 2, j=2
                        )
                        aP = (
                            a2[:, 2 * k * SC : 2 * (k + 1) * SC]
                            .rearrange("p (s j) -> p s j", j=2)
                            .unsqueeze(2)
                            .to_broadcast((C, SC, T // 2, 2))
                        )
                        nc.vector.tensor_tensor(oP, oP, aP, op=OP.mult)
                        # SWDGE cast fp16 -> f32 on the way out
                        nc.gpsimd.dma_start(out[b, :, sl, :], oc[:])

    nc.compile()
    return nc


def get_nc():
    global _NC
    if _NC is None:
        _NC = build_nc()
    return _NC


def shard_inputs(x, Wspect, Wtemp):
    ws = np.ascontiguousarray(Wspect.reshape(C, T).astype(np.float32))
    wt = np.ascontiguousarray(Wtemp.reshape(C, S).astype(np.float32))
    x = np.ascontiguousarray(x.astype(np.float32))
    return [
        {"x": x[i * B_LOC : (i + 1) * B_LOC], "wspect": ws, "wtemp": wt}
        for i in range(N_CORES)
    ]


def unshard(results):
    return np.concatenate([r["out"] for r in results], axis=0)


def kernel(x, Wspect, Wtemp):
    nc = get_nc()
    in_maps = shard_inputs(x, Wspect, Wtemp)
    res = run_bass_kernel_spmd(nc, in_maps, core_ids=list(range(N_CORES)))
    return unshard(res.results)


# revision 9
# speedup vs baseline: 1.7049x; 1.0012x over previous
"""Trainium2 Bass kernel for nn_Attention_59528246723073.

Reference (per batch b, channel c; x[b,c] is [S=256, T=64]):
    fs = tanh(x @ Wspect[c])            # [S]   (contract T)
    ft = tanh(x.T @ Wtemp[c])           # [T]   (contract S)
    a  = softmax_S(fs) * 100
    g  = softmax_T(ft)
    out[b,c,s,t] = x[b,c,s,t] * a[s] * g[t]

Distribution: data-parallel over batch B=32 -> 4 per core on 8 cores.

Per-core layout: for each local batch b, SBUF tile [128 part = channels,
S*T free] (x[b] is exactly this, contiguous).  f32 chunks are cast to fp16
on ScalarE; all big elementwise ops run on VectorE in fp16 with the 2x_1p
perf mode (innermost step 1 on every operand):
  - fs-mul multiplies by Wspect broadcast over s (inner t contiguous),
  - ft-mul multiplies by a pre-materialized Wtemp replica (contiguous, flat),
  - ft reduction = flat in-place fold chain over s, accumulated across chunks,
  - fs reduction = one in-place fold over t + one f32 tensor_reduce,
  - final: g-mul (inner-contiguous bcast) then a-mul via a paired-duplicate
    a2[p, 2s+j] = a[p,s] so the broadcast keeps innermost step 1.
Output stays fp16 in SBUF and is cast to f32 by the SWDGE output DMA.
"""

import numpy as np

import concourse.bass as bass
import concourse.tile as tile
from concourse import bacc, mybir
from concourse.bass_utils import run_bass_kernel_spmd

B, C, S, T = 32, 128, 256, 64
N_CORES = 8
B_LOC = B // N_CORES
NCHUNK = 4
SC = S // NCHUNK  # s-values per chunk
F32 = mybir.dt.float32
F16 = mybir.dt.float16

_NC = None


def build_nc():
    nc = bacc.Bacc("TRN2", target_bir_lowering=False, debug=False)
    x = nc.dram_tensor("x", [B_LOC, C, S, T], F32, kind="ExternalInput")
    ws = nc.dram_tensor("wspect", [C, T], F32, kind="ExternalInput")
    wt = nc.dram_tensor("wtemp", [C, S], F32, kind="ExternalInput")
    out = nc.dram_tensor("out", [B_LOC, C, S, T], F32, kind="ExternalOutput")

    AF = mybir.ActivationFunctionType
    OP = mybir.AluOpType
    AX = mybir.AxisListType

    with tile.TileContext(nc) as tc:
        with (
            tc.tile_pool(name="consts", bufs=1) as cpool,
            tc.tile_pool(name="x2", bufs=2) as x2pool,
            tc.tile_pool(name="tmp", bufs=4) as tpool,
            tc.tile_pool(name="small", bufs=2) as spool,
        ):
            # --- constants: weights in fp16; Wtemp replicated along t ---
            ws_f = cpool.tile([C, T], F32)
            nc.sync.dma_start(ws_f[:], ws[:])
            wt_f = cpool.tile([C, S], F32)
            nc.sync.dma_start(wt_f[:], wt[:])
            ws16 = cpool.tile([C, T], F16)
            nc.scalar.activation(ws16[:], ws_f[:], AF.Copy)
            # wt_rep[c, s, t] = Wtemp[c, s]  (fp16, contiguous)
            wt_rep = cpool.tile([C, S * T], F16)
            wt_rep3 = wt_rep.rearrange("p (s t) -> p s t", t=T)
            nc.scalar.activation(
                wt_rep3, wt_f[:].unsqueeze(2).to_broadcast((C, S, T)), AF.Copy
            )
            # [128, s-chunk(bcast), t] — inner step 1 -> 2x
            ws_bc = ws16.unsqueeze(1).to_broadcast((C, SC, T))

            for b in range(B_LOC):
                X2 = x2pool.tile([C, S * T], F16, tag="X2")
                X23 = X2.rearrange("p (s t) -> p s t", t=T)
                fs = spool.tile([C, S], F32, tag="fs")
                ft = spool.tile([C, T], F32, tag="ft")

                for k in range(NCHUNK):
                    sl = slice(k * SC, (k + 1) * SC)
                    fsl = slice(k * SC * T, (k + 1) * SC * T)
                    # load chunk with SWDGE cast f32 -> fp16 during DMA
                    with nc.named_scope("load"):
                        nc.gpsimd.dma_start(X2[:, fsl], x[b, :, sl, :])

                    xc = X23[:, sl, :]
                    xcf = X2[:, fsl]
                    # fs[:, sl] = sum_t xc * Wspect[:, None, :]
                    with nc.named_scope("fs"):
                        tmp = tpool.tile([C, SC * T], F16, tag="tmp")
                        t3 = tmp.rearrange("p (s t) -> p s t", t=T)
                        nc.vector.tensor_tensor(t3, xc, ws_bc, op=OP.mult)
                        nc.vector.tensor_tensor(
                            t3[:, :, 0 : T // 2], t3[:, :, 0 : T // 2],
                            t3[:, :, T // 2 : T], op=OP.add,
                        )
                        nc.vector.tensor_tensor(
                            t3[:, :, 0 : T // 4], t3[:, :, 0 : T // 4],
                            t3[:, :, T // 4 : T // 2], op=OP.add,
                        )
                        nc.vector.reduce_sum(
                            fs[:, sl], t3[:, :, 0 : T // 4], axis=AX.X
                        )
                    # ft += sum_{s in chunk} xc * Wtemp[:, sl, None]
                    # (all-flat fp16 fold chain over s, then tiny accumulate)
                    with nc.named_scope("ft"):
                        tmp2 = tpool.tile([C, SC * T], F16, tag="tmp")
                        nc.vector.tensor_tensor(
                            tmp2[:], xcf, wt_rep[:, fsl], op=OP.mult
                        )
                        w = SC * T // 2
                        while w >= T:
                            nc.vector.tensor_tensor(
                                tmp2[:, 0:w], tmp2[:, 0:w], tmp2[:, w : 2 * w],
                                op=OP.add,
                            )
                            w //= 2
                        if k == 0:
                            nc.vector.tensor_copy(ft[:], tmp2[:, 0:T])
                        else:
                            nc.vector.tensor_tensor(
                                ft[:], ft[:], tmp2[:, 0:T], op=OP.add
                            )

                with nc.named_scope("softmax"):
                    nc.scalar.activation(fs[:], fs[:], AF.Tanh)
                    nc.scalar.activation(ft[:], ft[:], AF.Tanh)

                    # softmax over S -> a2 (fp16, paired-duplicate, x100)
                    nmax = spool.tile([C, 1], F32, tag="nmax")
                    ssum = spool.tile([C, 1], F32, tag="ssum")
                    rec = spool.tile([C, 1], F32, tag="rec")
                    nc.vector.reduce_max(nmax[:], fs[:], axis=AX.X, negate=True)
                    nc.scalar.activation(
                        fs[:], fs[:], AF.Exp, bias=nmax[:, 0:1],
                        accum_out=ssum[:, 0:1],
                    )
                    nc.vector.reciprocal(rec[:], ssum[:])
                    a2 = spool.tile([C, 2 * S], F16, tag="a2")
                    nc.vector.tensor_scalar(
                        out=a2.rearrange("p (s j) -> p s j", j=2),
                        in0=fs[:].unsqueeze(2).to_broadcast((C, S, 2)),
                        scalar1=rec[:, 0:1], scalar2=100.0,
                        op0=OP.mult, op1=OP.mult,
                    )

                    # softmax over T -> g16 (fp16)
                    nmax2 = spool.tile([C, 1], F32, tag="nmax2")
                    ssum2 = spool.tile([C, 1], F32, tag="ssum2")
                    rec2 = spool.tile([C, 1], F32, tag="rec2")
                    nc.vector.reduce_max(nmax2[:], ft[:], axis=AX.X, negate=True)
                    nc.scalar.activation(
                        ft[:], ft[:], AF.Exp, bias=nmax2[:, 0:1],
                        accum_out=ssum2[:, 0:1],
                    )
                    nc.vector.reciprocal(rec2[:], ssum2[:])
                    g16 = spool.tile([C, T], F16, tag="g16")
                    nc.vector.tensor_scalar(
                        out=g16[:], in0=ft[:], scalar1=rec2[:, 0:1], scalar2=None,
                        op0=OP.mult,
                    )
                g_bc = g16.unsqueeze(1).to_broadcast((C, SC, T))

                for k in range(NCHUNK):
                    sl = slice(k * SC, (k + 1) * SC)
                    with nc.named_scope("final"):
                        oc = tpool.tile([C, SC * T], F16, tag="tmp")
                        o3 = oc.rearrange("p (s t) -> p s t", t=T)
                        nc.vector.tensor_tensor(o3, X23[:, sl, :], g_bc, op=OP.mult)
                        # a-mul on fp16 pairs: innermost step-1 j dim keeps 2x
                        oP = oc.rearrange(
                            "p (s pr j) -> p s pr j", pr=T // 2, j=2
                        )
                        aP = (
                            a2[:, 2 * k * SC : 2 * (k + 1) * SC]
                            .rearrange("p (s j) -> p s j", j=2)
                            .unsqueeze(2)
                            .to_broadcast((C, SC, T // 2, 2))
                        )
                        nc.vector.tensor_tensor(oP, oP, aP, op=OP.mult)
                        # SWDGE cast fp16 -> f32 on the way out
                        nc.gpsimd.dma_start(out[b, :, sl, :], oc[:])

    nc.compile()
    return nc


def get_nc():
    global _NC
    if _NC is None:
        _NC = build_nc()
    return _NC


def shard_inputs(x, Wspect, Wtemp):
    ws = np.ascontiguousarray(Wspect.reshape(C, T).astype(np.float32))
    wt = np.ascontiguousarray(Wtemp.reshape(C, S).astype(np.float32))
    x = np.ascontiguousarray(x.astype(np.float32))
    return [
        {"x": x[i * B_LOC : (i + 1) * B_LOC], "wspect": ws, "wtemp": wt}
        for i in range(N_CORES)
    ]


def unshard(results):
    return np.concatenate([r["out"] for r in results], axis=0)


def kernel(x, Wspect, Wtemp):
    nc = get_nc()
    in_maps = shard_inputs(x, Wspect, Wtemp)
    res = run_bass_kernel_spmd(nc, in_maps, core_ids=list(range(N_CORES)))
    return unshard(res.results)
